# revision 19
# baseline (speedup 1.0000x reference)
"""Contrastive-loss kernel for 8 Trainium2 NeuronCores.

Math (reference):
    sim = X @ X.T                               # [n, n]
    pos = targets[:,None] == targets[None,:]
    loss = ( sum(where(pos & sim<1,  1-sim, 0))
           + sum(where(~pos & sim>m, sim,  0)) ) / n    with m = 0.3

Device decomposition (per element s of sim, with a = relu(1-s),
u = relu(s-m), c = m*step(s-m), z = a - u - c):
    f_neg(s) = s*step(s-m) = u + c
    f_pos(s) = a
    total = sum_all(u) + sum_all(c) + sum_pos(z)
The diagonal (i==j) is a "pos" pair and cancels exactly: z + u + c = a = 0
for s ~ ||x||^2 >> 1.

Sharding: data-parallel over rows. Core r computes the [8192, 1024] block
T[j, i] = <x_j, x_i> for its 1024 local columns i, as 64 j-tiles of
[128, 1024] via bf16 PE matmuls (K=512 contracted in 4 chunks of 128) from
a host-pretransposed XT = X.T.  sum_pos(z) is evaluated without ever
materializing the [n, n] label mask: per j-tile the PE also computes
P_j.T @ z into a persistent PSUM accumulator ([128 classes, 1024 i],
accumulated over all 64 j-tiles), which at the end is reduced against
P_local.T (one-hot of local labels) on the DVE.  sum(u) / sum(c) row-sums
ride for free on the ACT / DVE ops that produce u and c.

Host does: transpose + bf16 cast of X, one-hot of targets, final sum of
8 x [128, 3] partials.
"""

import numpy as np
import ml_dtypes

N = 8192
D = 512
C = 128          # number of classes
NCORES = 8
NL = N // NCORES  # local columns per core (1024)
KT = D // 128     # k tiles (4)
NT = N // 128     # j tiles (64)
NCHUNK = 4        # xt free-dim chunks
CHW = N // NCHUNK  # chunk width (2048)
JT_PER_CHUNK = NT // NCHUNK  # 16
MARGIN = 0.3

_BF16 = ml_dtypes.bfloat16

_COMPILED = None     # cached (nc,) so repeat kernel() calls skip rebuild
LAST_RESULTS = None  # BassKernelResults of the most recent run (for profiling)


def _build():
    import concourse.tile as tile
    from concourse import bacc, mybir

    nc = bacc.Bacc("TRN2", target_bir_lowering=False, debug=False,
                   num_devices=NCORES)
    bf16 = mybir.dt.bfloat16
    f32 = mybir.dt.float32

    xt_d = nc.dram_tensor("xt", [D, N], bf16, kind="ExternalInput").ap()
    xtl_d = nc.dram_tensor("xt_loc", [D, NL], bf16, kind="ExternalInput").ap()
    p_d = nc.dram_tensor("p", [N, C], bf16, kind="ExternalInput").ap()
    p3_d = nc.dram_tensor("p3", [N, C], bf16, kind="ExternalInput").ap()
    plt_d = nc.dram_tensor("ploc_t", [C, NL], bf16, kind="ExternalInput").ap()
    out_d = nc.dram_tensor("out", [128, 4], f32, kind="ExternalOutput").ap()

    with tile.TileContext(nc) as tc:
        with (
            tc.tile_pool(name="xt", bufs=1) as xt_pool,
            tc.tile_pool(name="xtl", bufs=1) as xtl_pool,
            tc.tile_pool(name="pp", bufs=1) as p_pool,
            tc.tile_pool(name="acc", bufs=1) as acc_pool,
            tc.tile_pool(name="work", bufs=3) as work,
            tc.tile_pool(name="psum_s", bufs=3, space="PSUM") as psum_s_pool,
            tc.tile_pool(name="psum_p", bufs=1, space="PSUM") as psum_p_pool,
        ):
            # -- resident inputs ------------------------------------------
            xtl_sb = []
            for kt in range(KT):
                t = xtl_pool.tile([128, NL], bf16, tag=f"xtl{kt}")
                nc.sync.dma_start(t[:], xtl_d[kt * 128:(kt + 1) * 128, :])
                xtl_sb.append(t)

            p_sb = p_pool.tile([128, NT, C], bf16)
            p_view = p_d.rearrange("(t p) c -> p t c", p=128)
            for tch in range(8):
                nc.sync.dma_start(
                    p_sb[:, tch * 8:(tch + 1) * 8, :],
                    p_view[:, tch * 8:(tch + 1) * 8, :],
                )
            # p3 = -bf16(0.3) * P, merges the 0.3*step correction into the
            # same PSUM accumulator as the z2 projection
            p3_sb = p_pool.tile([128, NT, C], bf16)
            p3_view = p3_d.rearrange("(t p) c -> p t c", p=128)
            for tch in range(8):
                nc.sync.dma_start(
                    p3_sb[:, tch * 8:(tch + 1) * 8, :],
                    p3_view[:, tch * 8:(tch + 1) * 8, :],
                )

            # xt in column chunks so PE can start before the full 8MB lands
            xt_sb = [[None] * NCHUNK for _ in range(KT)]
            for ch in range(NCHUNK):
                for kt in range(KT):
                    t = xt_pool.tile([128, CHW], bf16, tag=f"xt{kt}_{ch}")
                    nc.sync.dma_start(
                        t[:],
                        xt_d[kt * 128:(kt + 1) * 128,
                             ch * CHW:(ch + 1) * CHW],
                    )
                    xt_sb[kt][ch] = t

            plt_sb = acc_pool.tile([C, NL], bf16)
            nc.sync.dma_start(plt_sb[:], plt_d[:])

            # -- persistent accumulators ----------------------------------
            accu = acc_pool.tile([128, NT], f32)   # per-j-tile row sums of u
            accc = acc_pool.tile([128, NT], f32)   # per-j-tile counts of c
            # accumulates sum_j (P_j.T @ z2 - 0.30078125 * P_j.T @ c)
            psum_projz = psum_p_pool.tile([128, NL], f32)

            bias_m = acc_pool.tile([128, 1], f32)  # ACT bias for relu(s - m)
            nc.vector.memset(bias_m[:], -MARGIN)

            relu = mybir.ActivationFunctionType.Relu
            alu = mybir.AluOpType

            for jt in range(NT):
                ch, off = jt // JT_PER_CHUNK, (jt % JT_PER_CHUNK) * 128

                # s tile: [128 j, 1024 i] f32 in PSUM
                psum_s = psum_s_pool.tile([128, NL], f32, tag="psum_s")
                for h in range(2):
                    for kt in range(KT):
                        nc.tensor.matmul(
                            psum_s[:, h * 512:(h + 1) * 512],
                            lhsT=xt_sb[kt][ch][:, off:off + 128],
                            rhs=xtl_sb[kt][:, h * 512:(h + 1) * 512],
                            start=(kt == 0),
                            stop=(kt == KT - 1),
                        )

                a_sb = work.tile([128, NL], bf16, tag="a")
                nc.scalar.activation(a_sb[:], psum_s[:], relu,
                                     bias=1.0, scale=-1.0)
                u_sb = work.tile([128, NL], bf16, tag="u")
                nc.scalar.activation(u_sb[:], psum_s[:], relu,
                                     bias=bias_m[:], scale=1.0,
                                     accum_out=accu[:, jt:jt + 1])
                # c = step(s - m) as 0/1 bf16; accum_out gets the row count
                # (op1 is the REDUCTION op when accum_out is present)
                c_sb = work.tile([128, NL], bf16, tag="c")
                nc.vector.tensor_scalar(c_sb[:], u_sb[:], 0.0, None,
                                        op0=alu.is_gt, op1=alu.add,
                                        accum_out=accc[:, jt:jt + 1])
                z_sb = work.tile([128, NL], bf16, tag="z")
                nc.vector.tensor_tensor(z_sb[:], a_sb[:], u_sb[:],
                                        op=alu.subtract)

                for h in range(2):
                    nc.tensor.matmul(
                        psum_projz[:, h * 512:(h + 1) * 512],
                        lhsT=p_sb[:, jt, :],
                        rhs=z_sb[:, h * 512:(h + 1) * 512],
                        start=(jt == 0),
                        stop=False,
                    )
                    nc.tensor.matmul(
                        psum_projz[:, h * 512:(h + 1) * 512],
                        lhsT=p3_sb[:, jt, :],
                        rhs=c_sb[:, h * 512:(h + 1) * 512],
                        start=False,
                        stop=(jt == NT - 1),
                    )

            # -- final reduction ------------------------------------------
            out_sb = acc_pool.tile([128, 4], f32)
            nc.vector.reduce_sum(out_sb[:, 0:1], accu[:],
                                 axis=mybir.AxisListType.X)
            nc.vector.reduce_sum(out_sb[:, 1:2], accc[:],
                                 axis=mybir.AxisListType.X)
            junk = acc_pool.tile([128, NL], f32)
            nc.vector.tensor_tensor(junk[:], psum_projz[:], plt_sb[:],
                                    op=alu.mult)
            nc.vector.reduce_sum(out_sb[:, 2:3], junk[:],
                                 axis=mybir.AxisListType.X)
            nc.vector.memset(out_sb[:, 3:4], 0.0)
            nc.sync.dma_start(out_d[:], out_sb[:])

    nc.compile()
    return nc


def kernel(inputs, targets):
    global _COMPILED, LAST_RESULTS
    from concourse.bass_utils import run_bass_kernel_spmd

    X = np.asarray(inputs, dtype=np.float32)
    t = np.asarray(targets).astype(np.int64)
    assert X.shape == (N, D) and t.shape == (N,)

    XT = np.ascontiguousarray(X.astype(_BF16).T)            # [512, 8192]
    P = (t[:, None] == np.arange(C)[None, :]).astype(_BF16)  # [8192, 128]
    # -bf16(0.3) * P; 0.30078125 is exact in bf16 so P3 entries are exact
    M3 = np.float32(_BF16(MARGIN))
    P3 = (-M3 * P.astype(np.float32)).astype(_BF16)

    if _COMPILED is None:
        _COMPILED = _build()
    nc = _COMPILED

    in_maps = []
    for r in range(NCORES):
        sl = slice(r * NL, (r + 1) * NL)
        in_maps.append({
            "xt": XT,
            "xt_loc": np.ascontiguousarray(XT[:, sl]),
            "p": P,
            "p3": P3,
            "ploc_t": np.ascontiguousarray(P[sl].T),
        })

    res = run_bass_kernel_spmd(nc, in_maps, list(range(NCORES)))
    LAST_RESULTS = res

    # out cols: [sum(u), count(c), sum_pos(a-u) - bf16(m)*count_pos(c), 0]
    # total = sum(u) + m*count(c) + col2
    m64 = np.float64(np.float32(MARGIN))
    total = np.float64(0.0)
    for r in range(NCORES):
        cols = res.results[r]["out"].astype(np.float64).sum(axis=0)
        total += cols[0] + m64 * cols[1] + cols[2]
    return np.asarray(total / N, dtype=np.float32)
